# revision 1
# baseline (speedup 1.0000x reference)
"""YIN pitch Trainium2 kernel, Phase 2: PE band-matmul difference function.

C[f,tau] = sum_n x[n]*x[n+tau]*[80f <= n <= 80f+132] on the tensor engine:
contraction over 128-sample tiles (k = partition = sample), stationary
operand = x-valued band selector slab [128, 32] (<=4 active frame columns,
zero padded; slab positions repeat with period 20 tiles), moving operand =
Hankel slice XD[:, 128t+1 : 128t+134] where XD[p, c] = x[p+c], streamed from
a DRAM bounce buffer in fp8.  PSUM accumulates 32-frame windows (out rows
always [0, 32) - PE requires 32-aligned PSUM base partitions).

Energy terms + CMNDF threshold pick stay on DVE in f32.
"""

import math

import numpy as np

import bass_rust
import concourse.bass as bass
import concourse.mybir as mybir
import concourse.tile as tile
from concourse.bass_utils import run_bass_kernel_spmd
from concourse.tile_rust import add_dep_helper

_WAIT_LIM = 1


def _split_excess_waits(nc):
    uid = 0
    for fn in nc.m.functions:
        for blk in fn.blocks:
            out = []
            changed = False
            for inst in blk.instructions:
                si = inst.sync_info
                waits = list(si.on_wait) if si is not None and si.on_wait else []
                if len(waits) > _WAIT_LIM:
                    changed = True
                    extra = waits[:-_WAIT_LIM]
                    si.on_wait = waits[-_WAIT_LIM:]
                    for j in range(0, len(extra), _WAIT_LIM):
                        nop = bass_rust.InstNoOp(name=f"WSPLIT-{uid}", ins=[], outs=[])
                        uid += 1
                        nop.engine = inst.engine
                        nop.sync_info = bass_rust.SyncInfo(
                            on_wait=extra[j:j + _WAIT_LIM], on_update=[]
                        )
                        out.append(nop)
                out.append(inst)
            if changed:
                blk.instructions = out


def _short_drain_and_barrier(self, tick_clock, wait_clock):
    # Tail with a single all-engine barrier: drain, barrier, sem cleanup.
    # The trailing barrier of the stock TileContext tail only re-syncs
    # engines that have no further work; the runtime joins engines anyway.
    from concourse.vector_clock import ScopedClock
    nc = self.nc
    drain_inst = nc.sync.drain()
    wait_clock.add_sem_waits(
        drain_inst.ins, ScopedClock({None: tick_clock.global_clock})
    )
    nc.all_engine_barrier()
    assert self.sems is not None
    popped = nc._tile_sem_poison_stack.pop()
    assert popped is self._sem_poison
    nc.clear_and_free_semaphores(list(self.sems.allocated().values()))


tile.TileContext._drain_and_barrier = _short_drain_and_barrier


B = 8
N = 80000
SR = 8000
HOP = 80
TAU_MIN = 20
TAU_MAX = 133
W = 133
FRAME_LEN = 266
N_FRAMES = 997
N_OUT = 996          # frames 0..995 are emitted
THRESH = 0.2
EPS = 1e-8
BIG = 1.0e9

N_BLK = 8
FT = 268
G = 4                # max frames per 128-sample tile
NT = 625             # sample tiles
NCHUNK = 640         # xpad chunk width: [128, 640] covers 81920 samples
SEG_T = 128          # tiles per XD segment
SEG_LEN = SEG_T * 128 + TAU_MAX    # 3333
N_SEG = 5
WIN = 32             # frames per PSUM window
PERIOD = 20          # slab-position periodicity in tiles

F32 = mybir.dt.float32
BF16 = mybir.dt.bfloat16
DT_LOW = mybir.dt.float8e4   # PE operand dtype (e4m3); set BF16 to fall back
AluOp = mybir.AluOpType
Axis = mybir.AxisListType


def _ap(t, offset, pairs):
    return bass.AP(t, offset, pairs)


def _sap(tile_ap, offset, pairs):
    """AP on an SBUF tile: partition pair step = row pitch (elements)."""
    pitch = tile_ap[:, 0:1].ap[0][0]
    return bass.AP(tile_ap.tensor, offset, [[pitch, pairs[0][1]]] + pairs[1:])


def _fb(t):
    return math.ceil((128 * t - (W - 1)) / HOP)


def _geometry():
    """Period-5 cover mask + period-20 slab groups."""
    mask5 = np.zeros((128, 5, G), np.float32)
    for r in range(5):
        n0 = 128 * r
        fb = _fb(r)
        for g in range(G):
            f = fb + g
            lo = max(0, HOP * f - n0)
            hi = min(127, HOP * f + (W - 1) - n0)
            if lo <= hi:
                mask5[lo:hi + 1, r, g] = 1.0

    t_eff = max(t for t in range(NT) if _fb(t) <= N_OUT - 1)
    groups = []
    for rho in range(PERIOD):
        fb = _fb(rho)
        byw = {}
        for g in range(G):
            byw.setdefault((fb + g) // WIN, []).append(g)
        for a_off, gs in sorted(byw.items()):
            groups.append(
                dict(rho=rho, a_off=a_off, glo=min(gs), ghi=max(gs),
                     pos=(fb + min(gs)) - WIN * a_off)
            )
    return mask5, groups, t_eff


def _build_nc():
    nc = bass.Bass(trn_type="TRN2")
    x_d = nc.dram_tensor("x", [N], F32, kind="ExternalInput")
    f0_d = nc.dram_tensor("f0", [N_OUT], F32, kind="ExternalOutput")

    mask5, groups, t_eff = _geometry()
    n_groups = len(groups)
    for gi, gr in enumerate(groups):
        gr["nv"] = (t_eff - gr["rho"]) // PERIOD + 1
        gr["gi"] = gi
    by_rho = {}
    for gr in groups:
        by_rho.setdefault(gr["rho"], []).append(gr)

    tau_row = np.arange(1, TAU_MAX + 1, dtype=np.float32)
    tauc_d = nc.inline_tensor(np.broadcast_to(tau_row, (128, W)).copy(), name="tauc")
    taubig_d = nc.inline_tensor(
        (np.broadcast_to(tau_row, (128, W)) + np.float32(BIG)).astype(np.float32),
        name="taubig",
    )
    ident_d = nc.inline_tensor(np.eye(128, dtype=np.float32), name="ident")
    mask_d = nc.inline_tensor(
        mask5.reshape(128, 5 * G).astype(np.dtype(mybir.dt.np(BF16))), name="bmask"
    )
    zl_d = nc.inline_tensor(
        np.zeros((1, WIN), dtype=np.dtype(mybir.dt.np(DT_LOW))), name="zl"
    )
    zr_d = nc.inline_tensor(
        np.zeros((1, W), dtype=np.dtype(mybir.dt.np(DT_LOW))), name="zr"
    )

    # pieces per tile -> windows per pair; win_last in pair units
    def _pieces(t):
        fb = _fb(t)
        byw = {}
        for g in range(G):
            byw.setdefault((fb + g) // WIN, []).append(g)
        return [
            dict(a=a, glo=min(gs), ghi=max(gs), pos=(fb + min(gs)) - WIN * a)
            for a, gs in sorted(byw.items())
        ]

    pair_wins = {}
    win_last = {}
    for t2 in range(313):
        wins = set()
        for t in (2 * t2, 2 * t2 + 1):
            if t > t_eff:
                continue
            for pc in _pieces(t):
                f_lo = max(_fb(t) + pc["glo"], 0)
                f_hi = min(_fb(t) + pc["ghi"], N_OUT - 1)
                if f_lo <= f_hi and pc["a"] >= 0:
                    wins.add(pc["a"])
        if wins:
            pair_wins[t2] = tuple(sorted(wins))
            for a in wins:
                win_last[a] = t2

    with tile.TileContext(nc) as tc:
        with (
            tc.tile_pool(name="persist", bufs=1) as pp,
            tc.tile_pool(name="work", bufs=2) as wp,
            tc.tile_pool(name="xdpool", bufs=3) as xdp,
            tc.tile_pool(name="psum", bufs=6, space="PSUM") as psp,
            tc.tile_pool(name="ps2", bufs=1, space="PSUM") as ps2,
            tc.tile_pool(name="dram", bufs=1, space="DRAM") as dp,
        ):
            # ---- weight-slab zero fill first: overlaps the entire x chain
            xb = pp.tile([128, 640 * 64], DT_LOW)
            nc.gpsimd.memset(xb[:].bitcast(F32), 0.0)

            # ---- constants to SBUF
            tauc = pp.tile([128, W], F32)
            nc.scalar.dma_start(tauc[:], tauc_d[:])
            taubig = pp.tile([128, W], F32)
            nc.scalar.dma_start(taubig[:], taubig_d[:])
            ident = pp.tile([128, 128], F32)
            nc.scalar.dma_start(ident[:], ident_d[:])
            bmask = pp.tile([128, 5 * G], BF16)
            nc.sync.dma_start(bmask[:], mask_d[:])
            zl = pp.tile([1, WIN], DT_LOW)
            nc.scalar.dma_start(zl[:], zl_d[:])
            zr = pp.tile([1, W], DT_LOW)
            nc.scalar.dma_start(zr[:], zr_d[:])
            f0all = pp.tile([128, N_BLK], F32)
            nc.vector.memset(f0all[:], 0.0)

            # ---- x -> chunked SBUF (f32), convert, bounce to DRAM.
            # Order matters: the transpose-DMA switches the DMA xbar mode and
            # serializes against every in-flight DMACopy, so it runs before
            # the large copies.
            xchunk = pp.tile([128, NCHUNK], F32)
            nc.vector.memset(xchunk[:], 0.0)
            nc.sync.dma_start(
                xchunk[0:125, 0:NCHUNK],
                _ap(x_d, 0, [[NCHUNK, 125], [1, NCHUNK]]),
            )
            xbf = pp.tile([128, NCHUNK], BF16)
            nc.vector.tensor_copy(xbf[:], xchunk[:])
            xpad16_d = dp.tile([128, NCHUNK], BF16)
            nc.sync.dma_start(xpad16_d[:], xbf[:])
            xpm16 = pp.tile([128, NCHUNK], BF16)
            _tr = nc.sync.dma_start(
                xpm16[:],
                _ap(xpad16_d.tensor, 0, [[128, NCHUNK], [1, 128]]),
                transpose=True,
            )
            xlow = pp.tile([128, NCHUNK], DT_LOW)
            nc.vector.tensor_copy(xlow[:], xchunk[:])
            xpad8_d = dp.tile([128, NCHUNK], DT_LOW)
            nc.sync.dma_start(xpad8_d[:], xlow[:])

            # ---- weight slabs, t-major with window-parity slots:
            # tile t, window a piece -> cols [64 t + 32 (a%2) + pos, +ncols)
            for gr in groups:
                rho, nv = gr["rho"], gr["nv"]
                ncols = gr["ghi"] - gr["glo"] + 1
                for phi in (0, 1):  # v parity (slot alternates with v)
                    nu = (nv - phi + 1) // 2
                    if nu <= 0:
                        continue
                    slot = (gr["a_off"] + phi) % 2
                    base = 64 * (PERIOD * phi + rho) + 32 * slot + gr["pos"]
                    nc.vector.tensor_tensor(
                        out=_sap(xb, base, [[1, 128], [128 * PERIOD, nu], [1, ncols]]),
                        in0=_sap(xpm16, PERIOD * phi + rho,
                                 [[1, 128], [2 * PERIOD, nu], [0, ncols]]),
                        in1=_sap(bmask, (rho % 5) * G + gr["glo"],
                                 [[1, 128], [0, nu], [1, ncols]]),
                        op=AluOp.mult,
                    )
                # clip frames < 0 or > N_OUT-1 (first/last slots only)
                for v in (0, nv - 1):
                    t = PERIOD * v + rho
                    if t > t_eff:
                        continue
                    slot = (gr["a_off"] + v) % 2
                    for g in range(gr["glo"], gr["ghi"] + 1):
                        f = _fb(t) + g
                        if 0 <= f <= N_OUT - 1:
                            continue
                        col = 64 * t + 32 * slot + gr["pos"] + (g - gr["glo"])
                        nc.vector.memset(
                            _sap(xb, col, [[1, 128], [1, 1]]), 0.0
                        )

            # ---- E-path tiles per block (f32)
            xfr = {}
            qq = {}
            for b in range(N_BLK):
                Rb = 128 if b < N_BLK - 1 else N_OUT - 128 * (N_BLK - 1)
                xfr[b] = wp.tile([128, FT], F32, tag=f"xfr{b}", name=f"xfr{b}")
                nc.scalar.dma_start(
                    xfr[b][:Rb, :],
                    _ap(x_d, HOP * 128 * b, [[HOP, Rb], [1, FT]]),
                )
                sq = wp.tile([128, FRAME_LEN], F32, tag="sq")
                nc.scalar.square(sq[:Rb, :], xfr[b][:Rb, :FRAME_LEN])
                qq[b] = wp.tile([128, FRAME_LEN], F32, tag=f"qq{b}", name=f"qq{b}")
                nc.vector.tensor_tensor_scan(
                    qq[b][:Rb, :], sq[:Rb, :], sq[:Rb, :], 0.0,
                    AluOp.add, AluOp.bypass,
                )

            csb = {}
            for b in range(N_BLK):
                csb[b] = wp.tile([128, W], F32, tag=f"csb{b}", name=f"csb{b}")
            blk_done = {b: 0 for b in range(N_BLK)}

            def finish_block(b):
                Rb = 128 if b < N_BLK - 1 else N_OUT - 128 * (N_BLK - 1)
                e2 = wp.tile([128, W], F32, tag="e2")
                nc.vector.tensor_sub(
                    e2[:Rb, :], qq[b][:Rb, W:FRAME_LEN], qq[b][:Rb, 0:W]
                )
                d = wp.tile([128, W], F32, tag="d")
                nc.vector.scalar_tensor_tensor(
                    out=d[:Rb, :], in0=csb[b][:Rb, :], scalar=-2.0, in1=e2[:Rb, :],
                    op0=AluOp.mult, op1=AluOp.add,
                )
                nc.vector.tensor_scalar_add(d[:Rb, :], d[:Rb, :], qq[b][:Rb, W - 1:W])
                cum = wp.tile([128, W], F32, tag="cum")
                nc.vector.tensor_tensor_scan(
                    cum[:Rb, :], d[:Rb, :], d[:Rb, :], 0.0, AluOp.add, AluOp.bypass
                )
                lhs = wp.tile([128, W], F32, tag="lhs")
                nc.vector.tensor_mul(lhs[:Rb, :], d[:Rb, :], tauc[:Rb, :])
                rhs = wp.tile([128, W], F32, tag="rhs")
                nc.vector.tensor_scalar(
                    out=rhs[:Rb, :], in0=cum[:Rb, :], scalar1=EPS, scalar2=THRESH,
                    op0=AluOp.max, op1=AluOp.mult,
                )
                cand = wp.tile([128, W], F32, tag="cand")
                nc.vector.tensor_tensor(
                    out=cand[:Rb, :], in0=lhs[:Rb, :], in1=rhs[:Rb, :], op=AluOp.is_lt
                )
                v = wp.tile([128, W], F32, tag="v")
                nc.vector.scalar_tensor_tensor(
                    out=v[:Rb, :], in0=cand[:Rb, :], scalar=-BIG, in1=taubig[:Rb, :],
                    op0=AluOp.mult, op1=AluOp.add,
                )
                tmin = wp.tile([128, 1], F32, tag="tmin")
                nc.vector.tensor_reduce(
                    tmin[:Rb, :], v[:Rb, TAU_MIN - 1:W], axis=Axis.X, op=AluOp.min
                )
                voi = wp.tile([128, 1], F32, tag="voi")
                nc.vector.tensor_scalar(
                    out=voi[:Rb, :], in0=tmin[:Rb, :], scalar1=BIG * 0.5,
                    scalar2=None, op0=AluOp.is_lt,
                )
                rec = wp.tile([128, 1], F32, tag="rec")
                nc.vector.reciprocal(rec[:Rb, :], tmin[:Rb, :])
                f0v = wp.tile([128, 1], F32, tag="f0v")
                nc.vector.tensor_scalar(
                    out=f0v[:Rb, :], in0=rec[:Rb, :], scalar1=float(SR),
                    scalar2=None, op0=AluOp.mult,
                )
                nc.vector.tensor_mul(f0all[:Rb, b:b + 1], f0v[:Rb, :], voi[:Rb, :])

            # ---- band matmuls over XD segments (fp8 DoubleRow pairs)
            cps = {}
            DR = mybir.MatmulPerfMode.DoubleRow
            # progressive segments: small first chunk so matmuls start early
            seg_bounds = [0, 64, 192, 320, 448, 544, 608, 640]
            for si in range(len(seg_bounds) - 1):
                t0 = seg_bounds[si]
                if t0 > t_eff:
                    break
                seg_t = seg_bounds[si + 1] - t0
                seg_len = min(seg_t * 128 + TAU_MAX,
                              128 * NCHUNK - 128 * t0 - 127)
                xd = xdp.tile([128, seg_len], DT_LOW, tag="xd")
                nc.sync.dma_start(
                    xd[:], _ap(xpad8_d.tensor, 128 * t0, [[1, 128], [1, seg_len]])
                )
                for t2 in range(t0 // 2, min((t0 + seg_t) // 2, 312 + 1)):
                    if 2 * t2 > t_eff:
                        break
                    off2 = 256 * t2 - 128 * t0
                    wins = pair_wins.get(t2, ())
                    for a in wins:
                        if a not in cps:
                            cps[a] = psp.tile([WIN, W], F32, tag="c", name=f"c{a}")
                            nc.tensor.matmul(
                                cps[a][:], zl[:], zr[:], start=True, stop=False,
                            )
                        nc.tensor.matmul(
                            cps[a][:],
                            _sap(xb, 128 * t2 + 32 * (a % 2),
                                 [[1, 128], [64, 2], [1, WIN]]),
                            _sap(xd, off2 + 1, [[1, 128], [128, 2], [1, W]]),
                            start=False,
                            stop=(t2 == win_last[a]),
                            perf_mode=DR,
                            skip_group_check=True,
                        )
                    for a in sorted(cps.keys()):
                        if win_last[a] <= t2:
                            b = (WIN * a) // 128
                            r0 = (WIN * a) % 128
                            nc.scalar.copy(csb[b][r0:r0 + WIN, :], cps[a][:])
                            del cps[a]
                            blk_done[b] += 1
                            if blk_done[b] == 4:
                                finish_block(b)

            # ---- gather f0: transpose [128, 8] -> [8, 128], DMA out
            f0t = ps2.tile([N_BLK, 128], F32)
            nc.tensor.transpose(f0t[:], f0all[:, 0:N_BLK], ident[:])
            f0sb = pp.tile([N_BLK, 128], F32)
            nc.scalar.copy(f0sb[:], f0t[:])
            for b in range(N_BLK):
                cnt = 128 if b < N_BLK - 1 else N_OUT - 128 * (N_BLK - 1)
                nc.sync.dma_start(
                    _ap(f0_d, 128 * b, [[1, cnt]]), f0sb[b:b + 1, 0:cnt]
                )

    _split_excess_waits(nc)
    return nc


_NC_CACHE = {}


def _get_nc():
    if "nc" not in _NC_CACHE:
        _NC_CACHE["nc"] = _build_nc()
    return _NC_CACHE["nc"]


def kernel(x: np.ndarray) -> np.ndarray:
    x = np.ascontiguousarray(np.asarray(x), dtype=np.float32)
    assert x.shape == (B, N), x.shape
    nc = _get_nc()
    in_maps = [{"x": x[i]} for i in range(B)]
    res = run_bass_kernel_spmd(nc, in_maps, core_ids=list(range(B)))
    out = np.stack([np.asarray(res.results[i]["f0"]).reshape(N_OUT) for i in range(B)])
    return out.astype(np.float32)



# revision 10
# speedup vs baseline: 1.3080x; 1.3080x over previous
"""YIN pitch Trainium2 kernel, Phase 3: P=80 band-matmul.

C[f,tau] = sum_n x[n]*x[n+tau]*[80f <= n <= 80f+132] on the tensor engine.
With 80-sample contraction tiles, HOP=80 divides the tile exactly: tile t
touches only frames {t-1, t} with a fixed per-tile mask (ones for frame t,
[s<53] for frame t-1).  The whole selector slab is built with a handful of
wide-AP DVE ops, and the Hankel moving operand shrinks from 128 to 80
partition-shifted rows (6.5 MB of fp8 DMA instead of 10.6 MB).

Frames accumulate in eight [128, W] PSUM tiles (one per 128-frame output
block, written in 32-row slices = windows).  DVE reads PSUM directly for
the CMNDF threshold pick; Pool takes part of the elementwise chain.
The whole pipeline runs from one fp8 copy of x (3x threshold margin).
"""

import numpy as np

import bass_rust
import concourse.bass as bass
import concourse.mybir as mybir
import concourse.tile as tile
from concourse.bass_utils import run_bass_kernel_spmd

_WAIT_LIM = 1


def _split_excess_waits(nc):
    uid = 0
    for fn in nc.m.functions:
        for blk in fn.blocks:
            out = []
            changed = False
            for inst in blk.instructions:
                si = inst.sync_info
                waits = list(si.on_wait) if si is not None and si.on_wait else []
                if len(waits) > _WAIT_LIM:
                    changed = True
                    extra = waits[:-_WAIT_LIM]
                    si.on_wait = waits[-_WAIT_LIM:]
                    for j in range(0, len(extra), _WAIT_LIM):
                        nop = bass_rust.InstNoOp(name=f"WSPLIT-{uid}", ins=[], outs=[])
                        uid += 1
                        nop.engine = inst.engine
                        nop.sync_info = bass_rust.SyncInfo(
                            on_wait=extra[j:j + _WAIT_LIM], on_update=[]
                        )
                        out.append(nop)
                out.append(inst)
            if changed:
                blk.instructions = out


def _short_drain_and_barrier(self, tick_clock, wait_clock):
    # Tail with a single all-engine barrier: drain, barrier, sem cleanup.
    from concourse.vector_clock import ScopedClock
    nc = self.nc
    drain_inst = nc.sync.drain()
    wait_clock.add_sem_waits(
        drain_inst.ins, ScopedClock({None: tick_clock.global_clock})
    )
    nc.all_engine_barrier()
    assert self.sems is not None
    popped = nc._tile_sem_poison_stack.pop()
    assert popped is self._sem_poison
    nc.clear_and_free_semaphores(list(self.sems.allocated().values()))


tile.TileContext._drain_and_barrier = _short_drain_and_barrier


B = 8
N = 80000
SR = 8000
HOP = 80
TAU_MIN = 20
W = 133
FRAME_LEN = 266
N_OUT = 996
N_BLK = 8
BIG = 1.0e9

P = 80                   # contraction tile height (samples per tile)
NT = 1000                # sample tiles
NPAIR = 499              # DR pairs with live frames: u = 0..498
NWIN = 32                # 32-frame windows covering frames 0..1023
# progressive Hankel segments (tile counts must be even)
SEG_BOUNDS = [0, 64, 208, 400, 592, 784, 920, 968, 1000]

F32 = mybir.dt.float32
BF16 = mybir.dt.bfloat16
FP8 = mybir.dt.float8e4
AluOp = mybir.AluOpType
Axis = mybir.AxisListType
DR = mybir.MatmulPerfMode.DoubleRow


def _ap(t, offset, pairs):
    return bass.AP(t, offset, pairs)


def _sap(tile_ap, offset, pairs):
    """AP on an SBUF tile: partition pair step = row pitch (elements)."""
    pitch = tile_ap[:, 0:1].ap[0][0]
    return bass.AP(tile_ap.tensor, offset, [[pitch, pairs[0][1]]] + pairs[1:])


def _build_nc():
    nc = bass.Bass(trn_type="TRN2")
    x_d = nc.dram_tensor("x", [N], F32, kind="ExternalInput")
    f0_d = nc.dram_tensor("f0", [N_OUT], F32, kind="ExternalOutput")

    tau_row = np.arange(1, W + 1, dtype=np.float32)
    cpk_np = np.concatenate(
        [
            np.broadcast_to(5.0 * tau_row, (128, W)),       # 5*tau (thresh folded)
            np.broadcast_to(BIG + tau_row, (128, W)),       # BIG + tau
        ],
        axis=1,
    ).astype(np.float32)
    cpk_d = nc.inline_tensor(cpk_np, name="cpk")
    bm_np = np.zeros((96, 2), np.dtype(mybir.dt.np(BF16)))
    bm_np[:53, 0] = 1.0      # frame t-1 mask within tile t
    bm_np[:P, 1] = 1.0       # frame t mask (all 80 samples)
    bmask_d = nc.inline_tensor(bm_np, name="bmask")

    with tile.TileContext(nc) as tc:
        with (
            tc.tile_pool(name="persist", bufs=1) as pp,
            tc.tile_pool(name="work", bufs=2) as wp,
            tc.tile_pool(name="xdpool", bufs=3) as xdp,
            tc.tile_pool(name="psum", bufs=6, space="PSUM") as psp,
            tc.tile_pool(name="dram", bufs=1, space="DRAM") as dp,
        ):
            # ---- slab storage: col 32t + (f mod 32); boundary (t=32a, f=t-1)
            # entries live in bslab to avoid window contamination.
            xb = pp.tile([P, 32 * 1024], FP8)
            bslab = pp.tile([P, 64 * 31], FP8)
            # split the zero fill across Pool/DVE/Act so it clears early
            nc.gpsimd.memset(xb[:, 0:12800].bitcast(F32), 0.0)

            # ---- constants
            cpk = pp.tile([128, 2 * W], F32)
            nc.scalar.dma_start(cpk[:], cpk_d[:])
            bmask = pp.tile([96, 2], BF16)
            nc.scalar.dma_start(bmask[:], bmask_d[:])
            tauc5 = cpk[:, 0:W]
            taubig = cpk[:, W:2 * W]

            # ---- x -> SBUF chunk (f32), convert to fp8 + bf16, bounce to DRAM
            xchunk = pp.tile([128, 640], F32)
            xlow = pp.tile([128, 640], FP8)
            xbf = pp.tile([128, 640], BF16)
            nc.vector.memset(xlow[96:128, :].bitcast(F32), 0.0)
            nc.vector.memset(xbf[96:128, :].bitcast(F32), 0.0)
            nc.sync.dma_start(
                xchunk[0:125, :], _ap(x_d, 0, [[640, 125], [1, 640]])
            )
            nc.vector.tensor_copy(xlow[0:125, :], xchunk[0:125, :])
            nc.vector.tensor_copy(xbf[0:125, :], xchunk[0:125, :])

            nc.scalar.memzero(xb[:, 22528:32768].bitcast(F32))
            nc.vector.memset(xb[:, 12800:22528].bitcast(BF16), 0.0)
            nc.vector.memset(bslab[:].bitcast(F32), 0.0)

            xpad8_d = dp.tile([130, 640], FP8)
            xpad16_d = dp.tile([130, 640], BF16)
            nc.sync.dma_start(xpad8_d[0:128, :], xlow[:])
            nc.sync.dma_start(
                _ap(xpad8_d.tensor, 81920, [[1, 640]]), xlow[127:128, :]
            )
            nc.sync.dma_start(xpad16_d[0:128, :], xbf[:])
            nc.sync.dma_start(
                _ap(xpad16_d.tensor, 81920, [[1, 48]]), xbf[127:128, 0:48]
            )
            # transposed view: xpm[s, t] = x16[80 t + s]
            xpm = pp.tile([128, 1024], BF16)
            nc.sync.dma_start(
                xpm[:], _ap(xpad16_d.tensor, 0, [[80, 1024], [1, 128]]),
                transpose=True,
            )

            # ---- slab build (DVE): boundary frame-t, bslab, then main runs
            nc.vector.tensor_tensor(
                out=_sap(xb, 0, [[1, P], [1024, 32], [1, 1]]),
                in0=_sap(xpm, 0, [[1, P], [32, 32], [0, 1]]),
                in1=_sap(bmask, 1, [[1, P], [0, 32], [1, 1]]),
                op=AluOp.mult,
            )
            nc.vector.tensor_tensor(
                out=_sap(bslab, 31, [[1, P], [64, 31], [1, 1]]),
                in0=_sap(xpm, 32, [[1, P], [32, 31], [0, 1]]),
                in1=_sap(bmask, 0, [[1, P], [0, 31], [1, 1]]),
                op=AluOp.mult,
            )
            # main runs: window a, tiles t = 32a + w (w in [1,32)), two adjacent
            # cols per tile (frames t-1, t); col = 1024 a + 33 w - 1.
            for par in (0, 1):          # a parity
                for ilo, ihi in ((0, 8), (8, 16)):
                    ni = ihi - ilo
                    base = 2048 * ilo + 1024 * par + 32
                    nc.vector.tensor_tensor(
                        out=_sap(xb, base, [[1, P], [2048, ni], [33, 31], [1, 2]]),
                        in0=_sap(xpm, 64 * ilo + 32 * par + 1,
                                 [[1, P], [64, ni], [1, 31], [0, 2]]),
                        in1=_sap(bmask, 0, [[1, P], [0, ni], [0, 31], [1, 2]]),
                        op=AluOp.mult,
                    )

            # ---- energy path: frames from the fp8 bounce, square + scan
            xfrall = pp.tile([128, 8 * FRAME_LEN], FP8)
            nc.scalar.dma_start(
                _sap(xfrall, 0, [[1, 128], [FRAME_LEN, 8], [1, FRAME_LEN]]),
                _ap(xpad8_d.tensor, 0, [[HOP, 128], [HOP * 128, 8], [1, FRAME_LEN]]),
            )
            sq = {}
            qq = {}
            e1h = {}
            for b in range(N_BLK):
                sq[b] = pp.tile([128, FRAME_LEN], F32, name=f"sq{b}")
                nc.scalar.square(
                    sq[b][:], xfrall[:, FRAME_LEN * b:FRAME_LEN * (b + 1)]
                )
                qq[b] = pp.tile([128, FRAME_LEN], F32, name=f"qq{b}")
                e1h[b] = pp.tile([128, 1], F32, name=f"e1h{b}")
            for b in range(N_BLK):
                nc.vector.tensor_tensor_scan(
                    qq[b][:], sq[b][:], sq[b][:], 0.0, AluOp.add, AluOp.bypass
                )
                # -e1/2: folded into the window retire as an Act bias
                nc.vector.tensor_scalar(
                    out=e1h[b][:], in0=qq[b][:, W - 1:W], scalar1=-0.5,
                    scalar2=None, op0=AluOp.mult,
                )

            f0all = pp.tile([128, N_BLK], F32)

            # ---- PSUM: one [32, W] tile per window (matmul out must sit at
            # PSUM partition base 0); Act copies stopped windows into csb.
            cps = {}
            csb = [pp.tile([128, W], F32, name=f"csb{b}") for b in range(N_BLK)]

            def _pwin(a):
                if a not in cps:
                    cps[a] = psp.tile([32, W], F32, tag="c", name=f"c{a}")
                return cps[a][:]

            def retire_window(a):
                b, q = a // 4, a % 4
                # csb = C - e1/2  (so d = -2*csb + e2 = e1 + e2 - 2C)
                nc.scalar.add(
                    csb[b][32 * q:32 * q + 32, :], cps[a][:],
                    e1h[b][32 * q:32 * q + 32, :],
                )
                del cps[a]

            def finish_block(b):
                qb = qq[b]
                e12 = wp.tile([128, W], F32, tag="e12")
                nc.gpsimd.tensor_sub(e12[:], qb[:, W:FRAME_LEN], qb[:, 0:W])
                d = wp.tile([128, W], F32, tag="d")
                nc.vector.scalar_tensor_tensor(
                    out=d[:], in0=csb[b][:], scalar=-2.0, in1=e12[:],
                    op0=AluOp.mult, op1=AluOp.add,
                )
                cum = wp.tile([128, W], F32, tag="cum")
                nc.vector.tensor_tensor_scan(
                    cum[:], d[:], d[:], 0.0, AluOp.add, AluOp.bypass
                )
                lhs = wp.tile([128, W], F32, tag="lhs")
                nc.gpsimd.tensor_mul(lhs[:], d[:], tauc5)
                cand = wp.tile([128, W], F32, tag="cand")
                nc.vector.tensor_tensor(
                    out=cand[:], in0=lhs[:], in1=cum[:], op=AluOp.is_lt
                )
                v = wp.tile([128, W], F32, tag="v")
                nc.vector.scalar_tensor_tensor(
                    out=v[:], in0=cand[:], scalar=-BIG, in1=taubig,
                    op0=AluOp.mult, op1=AluOp.add,
                )
                tmin = wp.tile([128, 1], F32, tag="tmin")
                nc.vector.tensor_reduce(
                    tmin[:], v[:, TAU_MIN - 1:W], axis=Axis.X, op=AluOp.min
                )
                voi = wp.tile([128, 1], F32, tag="voi")
                nc.vector.tensor_scalar(
                    out=voi[:], in0=tmin[:], scalar1=BIG * 0.5,
                    scalar2=None, op0=AluOp.is_lt,
                )
                rec = wp.tile([128, 1], F32, tag="rec")
                nc.vector.reciprocal(rec[:], tmin[:])
                nc.vector.scalar_tensor_tensor(
                    out=f0all[:, b:b + 1], in0=voi[:], scalar=float(SR),
                    in1=rec[:], op0=AluOp.mult, op1=AluOp.mult,
                )

            # ---- band matmuls over Hankel segments
            for si in range(len(SEG_BOUNDS) - 1):
                t0, t1 = SEG_BOUNDS[si], SEG_BOUNDS[si + 1]
                seg_len = P * (t1 - t0) + 54
                xd = xdp.tile([P, P * 192 + 54], FP8, tag="xd")
                nc.sync.dma_start(
                    xd[:, 0:seg_len],
                    _ap(xpad8_d.tensor, P * t0, [[1, P], [1, seg_len]]),
                )
                for u in range(t0 // 2, t1 // 2):
                    if u >= NPAIR:
                        break
                    off2 = 160 * u - P * t0
                    mv = _sap(xd, off2 + 1, [[1, P], [P, 2], [1, W]])
                    a = (2 * u + 1) // 32
                    if u > 0 and (2 * u) % 32 == 0:
                        nc.tensor.matmul(
                            _pwin(a - 1),
                            _sap(bslab, 64 * (a - 1), [[1, P], [32, 2], [1, 32]]),
                            mv,
                            start=False, stop=True,
                            perf_mode=DR, skip_group_check=True,
                        )
                        retire_window(a - 1)
                    nc.tensor.matmul(
                        _pwin(a),
                        _sap(xb, 64 * u, [[1, P], [32, 2], [1, 32]]),
                        mv,
                        start=(u == 16 * a), stop=(u == NPAIR - 1),
                        perf_mode=DR, skip_group_check=True,
                    )
                    if u > 0 and (2 * u) % 128 == 0:
                        finish_block(u // 64 - 1)
            retire_window(31)
            finish_block(7)

            # ---- output: strided DMA straight from f0all
            nc.sync.dma_start(
                _ap(f0_d, 0, [[1, 128], [128, 7]]),
                _sap(f0all, 0, [[1, 128], [1, 7]]),
            )
            nc.sync.dma_start(
                _ap(f0_d, 896, [[1, 100]]), f0all[0:100, 7:8]
            )

    _split_excess_waits(nc)
    return nc


_NC_CACHE = {}


def _get_nc():
    if "nc" not in _NC_CACHE:
        _NC_CACHE["nc"] = _build_nc()
    return _NC_CACHE["nc"]


def kernel(x: np.ndarray) -> np.ndarray:
    x = np.ascontiguousarray(np.asarray(x), dtype=np.float32)
    assert x.shape == (B, N), x.shape
    nc = _get_nc()
    in_maps = [{"x": x[i]} for i in range(B)]
    res = run_bass_kernel_spmd(nc, in_maps, core_ids=list(range(B)))
    out = np.stack([np.asarray(res.results[i]["f0"]).reshape(N_OUT) for i in range(B)])
    return out.astype(np.float32)


# revision 12
# speedup vs baseline: 1.3467x; 1.0296x over previous
"""YIN pitch Trainium2 kernel, Phase 3: P=80 band-matmul.

C[f,tau] = sum_n x[n]*x[n+tau]*[80f <= n <= 80f+132] on the tensor engine.
With 80-sample contraction tiles, HOP=80 divides the tile exactly: tile t
touches only frames {t-1, t} with a fixed per-tile mask (ones for frame t,
[s<53] for frame t-1).  The whole selector slab is built with a handful of
wide-AP DVE ops, and the Hankel moving operand shrinks from 128 to 80
partition-shifted rows (6.5 MB of fp8 DMA instead of 10.6 MB).

Frames accumulate in eight [128, W] PSUM tiles (one per 128-frame output
block, written in 32-row slices = windows).  DVE reads PSUM directly for
the CMNDF threshold pick; Pool takes part of the elementwise chain.
The whole pipeline runs from one fp8 copy of x (3x threshold margin).
"""

import numpy as np

import bass_rust
import concourse.bass as bass
import concourse.mybir as mybir
import concourse.tile as tile
from concourse.bass_utils import run_bass_kernel_spmd

_WAIT_LIM = 1


def _split_excess_waits(nc):
    uid = 0
    for fn in nc.m.functions:
        for blk in fn.blocks:
            out = []
            changed = False
            for inst in blk.instructions:
                si = inst.sync_info
                waits = list(si.on_wait) if si is not None and si.on_wait else []
                if len(waits) > _WAIT_LIM:
                    changed = True
                    extra = waits[:-_WAIT_LIM]
                    si.on_wait = waits[-_WAIT_LIM:]
                    for j in range(0, len(extra), _WAIT_LIM):
                        nop = bass_rust.InstNoOp(name=f"WSPLIT-{uid}", ins=[], outs=[])
                        uid += 1
                        nop.engine = inst.engine
                        nop.sync_info = bass_rust.SyncInfo(
                            on_wait=extra[j:j + _WAIT_LIM], on_update=[]
                        )
                        out.append(nop)
                out.append(inst)
            if changed:
                blk.instructions = out


def _short_drain_and_barrier(self, tick_clock, wait_clock):
    # Tail with a single all-engine barrier: drain, barrier, sem cleanup.
    from concourse.vector_clock import ScopedClock
    nc = self.nc
    drain_inst = nc.sync.drain()
    wait_clock.add_sem_waits(
        drain_inst.ins, ScopedClock({None: tick_clock.global_clock})
    )
    nc.all_engine_barrier()
    assert self.sems is not None
    popped = nc._tile_sem_poison_stack.pop()
    assert popped is self._sem_poison
    nc.clear_and_free_semaphores(list(self.sems.allocated().values()))


tile.TileContext._drain_and_barrier = _short_drain_and_barrier


B = 8
N = 80000
SR = 8000
HOP = 80
TAU_MIN = 20
W = 133
FRAME_LEN = 266
N_OUT = 996
N_BLK = 8
BIG = 1.0e9

P = 80                   # contraction tile height (samples per tile)
NT = 1000                # sample tiles
NPAIR = 499              # DR pairs with live frames: u = 0..498
NWIN = 32                # 32-frame windows covering frames 0..1023
# progressive Hankel segments (tile counts must be even)
SEG_BOUNDS = [0, 64, 208, 400, 592, 784, 920, 968, 1000]

F32 = mybir.dt.float32
BF16 = mybir.dt.bfloat16
FP8 = mybir.dt.float8e4
AluOp = mybir.AluOpType
Axis = mybir.AxisListType
DR = mybir.MatmulPerfMode.DoubleRow


def _ap(t, offset, pairs):
    return bass.AP(t, offset, pairs)


def _sap(tile_ap, offset, pairs):
    """AP on an SBUF tile: partition pair step = row pitch (elements)."""
    pitch = tile_ap[:, 0:1].ap[0][0]
    return bass.AP(tile_ap.tensor, offset, [[pitch, pairs[0][1]]] + pairs[1:])


def _build_nc():
    nc = bass.Bass(trn_type="TRN2")
    x_d = nc.dram_tensor("x", [N], F32, kind="ExternalInput")
    f0_d = nc.dram_tensor("f0", [N_OUT], F32, kind="ExternalOutput")

    tau_row = np.arange(1, W + 1, dtype=np.float32)
    cpk_np = np.concatenate(
        [
            np.broadcast_to(5.0 * tau_row, (128, W)),       # 5*tau (thresh folded)
            np.broadcast_to(BIG + tau_row, (128, W)),       # BIG + tau
        ],
        axis=1,
    ).astype(np.float32)
    cpk_d = nc.inline_tensor(cpk_np, name="cpk")
    bm_np = np.zeros((96, 2), np.dtype(mybir.dt.np(BF16)))
    bm_np[:53, 0] = 1.0      # frame t-1 mask within tile t
    bm_np[:P, 1] = 1.0       # frame t mask (all 80 samples)
    bmask_d = nc.inline_tensor(bm_np, name="bmask")

    with tile.TileContext(nc) as tc:
        with (
            tc.tile_pool(name="persist", bufs=1) as pp,
            tc.tile_pool(name="work", bufs=2) as wp,
            tc.tile_pool(name="xdpool", bufs=3) as xdp,
            tc.tile_pool(name="psum", bufs=6, space="PSUM") as psp,
            tc.tile_pool(name="dram", bufs=1, space="DRAM") as dp,
        ):
            # ---- slab storage: col 32t + (f mod 32); boundary (t=32a, f=t-1)
            # entries live in bslab to avoid window contamination.
            xb = pp.tile([P, 32 * 1024], FP8)
            bslab = pp.tile([P, 64 * 31], FP8)
            # split the zero fill across Pool/Act; DVE stays free for the
            # x -> fp8/bf16 convert chain that gates the DMA pipeline
            nc.gpsimd.memset(xb[:, 0:18176].bitcast(F32), 0.0)

            # ---- constants
            cpk = pp.tile([128, 2 * W], F32)
            nc.scalar.dma_start(cpk[:], cpk_d[:])
            bmask = pp.tile([96, 2], BF16)
            nc.scalar.dma_start(bmask[:], bmask_d[:])
            tauc5 = cpk[:, 0:W]
            taubig = cpk[:, W:2 * W]

            # ---- x -> SBUF chunk (f32), convert to fp8 + bf16, bounce to DRAM
            xchunk = pp.tile([128, 640], F32)
            xlow = pp.tile([128, 640], FP8)
            xbf = pp.tile([128, 640], BF16)
            nc.vector.memset(xlow[96:128, :].bitcast(F32), 0.0)
            nc.vector.memset(xbf[96:128, :].bitcast(F32), 0.0)
            nc.sync.dma_start(
                xchunk[0:125, :], _ap(x_d, 0, [[640, 125], [1, 640]])
            )
            nc.vector.tensor_copy(xlow[0:125, :], xchunk[0:125, :])
            nc.vector.tensor_copy(xbf[0:125, :], xchunk[0:125, :])

            nc.scalar.memzero(xb[:, 18176:32768].bitcast(F32))
            nc.vector.memset(bslab[:].bitcast(F32), 0.0)

            xpad8_d = dp.tile([130, 640], FP8)
            xpad16_d = dp.tile([130, 640], BF16)
            nc.sync.dma_start(xpad8_d[0:128, :], xlow[:])
            nc.sync.dma_start(
                _ap(xpad8_d.tensor, 81920, [[1, 640]]), xlow[127:128, :]
            )
            nc.sync.dma_start(xpad16_d[0:128, :], xbf[:])
            nc.sync.dma_start(
                _ap(xpad16_d.tensor, 81920, [[1, 48]]), xbf[127:128, 0:48]
            )
            # transposed view: xpm[s, t] = x16[80 t + s]
            xpm = pp.tile([128, 1024], BF16)
            nc.sync.dma_start(
                xpm[:], _ap(xpad16_d.tensor, 0, [[80, 1024], [1, 128]]),
                transpose=True,
            )

            # ---- slab build (DVE): boundary frame-t, bslab, then main runs
            nc.vector.tensor_tensor(
                out=_sap(xb, 0, [[1, P], [1024, 32], [1, 1]]),
                in0=_sap(xpm, 0, [[1, P], [32, 32], [0, 1]]),
                in1=_sap(bmask, 1, [[1, P], [0, 32], [1, 1]]),
                op=AluOp.mult,
            )
            nc.vector.tensor_tensor(
                out=_sap(bslab, 31, [[1, P], [64, 31], [1, 1]]),
                in0=_sap(xpm, 32, [[1, P], [32, 31], [0, 1]]),
                in1=_sap(bmask, 0, [[1, P], [0, 31], [1, 1]]),
                op=AluOp.mult,
            )
            # main runs: window a, tiles t = 32a + w (w in [1,32)), two adjacent
            # cols per tile (frames t-1, t); col = 1024 a + 33 w - 1.
            for par in (0, 1):          # a parity
                for ilo, ihi in ((0, 8), (8, 16)):
                    ni = ihi - ilo
                    base = 2048 * ilo + 1024 * par + 32
                    nc.vector.tensor_tensor(
                        out=_sap(xb, base, [[1, P], [2048, ni], [33, 31], [1, 2]]),
                        in0=_sap(xpm, 64 * ilo + 32 * par + 1,
                                 [[1, P], [64, ni], [1, 31], [0, 2]]),
                        in1=_sap(bmask, 0, [[1, P], [0, ni], [0, 31], [1, 2]]),
                        op=AluOp.mult,
                    )

            # ---- energy path: frames from the fp8 bounce, square + scan
            xfrall = pp.tile([128, 8 * FRAME_LEN], FP8)
            nc.scalar.dma_start(
                _sap(xfrall, 0, [[1, 128], [FRAME_LEN, 8], [1, FRAME_LEN]]),
                _ap(xpad8_d.tensor, 0, [[HOP, 128], [HOP * 128, 8], [1, FRAME_LEN]]),
            )
            sq = {}
            qq = {}
            e1h = {}
            for b in range(N_BLK):
                sq[b] = pp.tile([128, FRAME_LEN], F32, name=f"sq{b}")
                nc.scalar.square(
                    sq[b][:], xfrall[:, FRAME_LEN * b:FRAME_LEN * (b + 1)]
                )
                qq[b] = pp.tile([128, FRAME_LEN], F32, name=f"qq{b}")
                e1h[b] = pp.tile([128, 1], F32, name=f"e1h{b}")
            for b in range(N_BLK):
                nc.vector.tensor_tensor_scan(
                    qq[b][:], sq[b][:], sq[b][:], 0.0, AluOp.add, AluOp.bypass
                )
                # -e1/2: folded into the window retire as an Act bias
                nc.vector.tensor_scalar(
                    out=e1h[b][:], in0=qq[b][:, W - 1:W], scalar1=-0.5,
                    scalar2=None, op0=AluOp.mult,
                )

            f0all = pp.tile([128, N_BLK], F32)

            # ---- PSUM: one [32, W] tile per window (matmul out must sit at
            # PSUM partition base 0); Act copies stopped windows into csb.
            cps = {}
            csb = [pp.tile([128, W], F32, name=f"csb{b}") for b in range(N_BLK)]

            def _pwin(a):
                if a not in cps:
                    cps[a] = psp.tile([32, W], F32, tag="c", name=f"c{a}")
                return cps[a][:]

            def retire_window(a):
                b, q = a // 4, a % 4
                # csb = C - e1/2  (so d = -2*csb + e2 = e1 + e2 - 2C)
                nc.scalar.add(
                    csb[b][32 * q:32 * q + 32, :], cps[a][:],
                    e1h[b][32 * q:32 * q + 32, :],
                )
                del cps[a]

            def finish_block(b):
                qb = qq[b]
                e12 = wp.tile([128, W], F32, tag="e12")
                nc.gpsimd.tensor_sub(e12[:], qb[:, W:FRAME_LEN], qb[:, 0:W])
                d = wp.tile([128, W], F32, tag="d")
                nc.vector.scalar_tensor_tensor(
                    out=d[:], in0=csb[b][:], scalar=-2.0, in1=e12[:],
                    op0=AluOp.mult, op1=AluOp.add,
                )
                cum = wp.tile([128, W], F32, tag="cum")
                nc.vector.tensor_tensor_scan(
                    cum[:], d[:], d[:], 0.0, AluOp.add, AluOp.bypass
                )
                lhs = wp.tile([128, W], F32, tag="lhs")
                nc.gpsimd.tensor_mul(lhs[:], d[:], tauc5)
                cand = wp.tile([128, W], F32, tag="cand")
                nc.vector.tensor_tensor(
                    out=cand[:], in0=lhs[:], in1=cum[:], op=AluOp.is_lt
                )
                v = wp.tile([128, W], F32, tag="v")
                nc.vector.scalar_tensor_tensor(
                    out=v[:], in0=cand[:], scalar=-BIG, in1=taubig,
                    op0=AluOp.mult, op1=AluOp.add,
                )
                tmin = wp.tile([128, 1], F32, tag="tmin")
                nc.vector.tensor_reduce(
                    tmin[:], v[:, TAU_MIN - 1:W], axis=Axis.X, op=AluOp.min
                )
                voi = wp.tile([128, 1], F32, tag="voi")
                nc.vector.tensor_scalar(
                    out=voi[:], in0=tmin[:], scalar1=BIG * 0.5,
                    scalar2=None, op0=AluOp.is_lt,
                )
                rec = wp.tile([128, 1], F32, tag="rec")
                nc.vector.reciprocal(rec[:], tmin[:])
                nc.vector.scalar_tensor_tensor(
                    out=f0all[:, b:b + 1], in0=voi[:], scalar=float(SR),
                    in1=rec[:], op0=AluOp.mult, op1=AluOp.mult,
                )

            # ---- band matmuls over Hankel segments
            for si in range(len(SEG_BOUNDS) - 1):
                t0, t1 = SEG_BOUNDS[si], SEG_BOUNDS[si + 1]
                seg_len = P * (t1 - t0) + 54
                xd = xdp.tile([P, P * 192 + 54], FP8, tag="xd")
                nc.sync.dma_start(
                    xd[:, 0:seg_len],
                    _ap(xpad8_d.tensor, P * t0, [[1, P], [1, seg_len]]),
                )
                for u in range(t0 // 2, t1 // 2):
                    if u >= NPAIR:
                        break
                    off2 = 160 * u - P * t0
                    mv = _sap(xd, off2 + 1, [[1, P], [P, 2], [1, W]])
                    a = (2 * u + 1) // 32
                    if u > 0 and (2 * u) % 32 == 0:
                        nc.tensor.matmul(
                            _pwin(a - 1),
                            _sap(bslab, 64 * (a - 1), [[1, P], [32, 2], [1, 32]]),
                            mv,
                            start=False, stop=True,
                            perf_mode=DR, skip_group_check=True,
                        )
                        retire_window(a - 1)
                    nc.tensor.matmul(
                        _pwin(a),
                        _sap(xb, 64 * u, [[1, P], [32, 2], [1, 32]]),
                        mv,
                        start=(u == 16 * a), stop=(u == NPAIR - 1),
                        perf_mode=DR, skip_group_check=True,
                    )
                    if u > 0 and (2 * u) % 128 == 0:
                        finish_block(u // 64 - 1)
            retire_window(31)
            finish_block(7)

            # ---- output: strided DMA straight from f0all
            nc.sync.dma_start(
                _ap(f0_d, 0, [[1, 128], [128, 7]]),
                _sap(f0all, 0, [[1, 128], [1, 7]]),
            )
            nc.sync.dma_start(
                _ap(f0_d, 896, [[1, 100]]), f0all[0:100, 7:8]
            )

    _split_excess_waits(nc)
    return nc


_NC_CACHE = {}


def _get_nc():
    if "nc" not in _NC_CACHE:
        _NC_CACHE["nc"] = _build_nc()
    return _NC_CACHE["nc"]


def kernel(x: np.ndarray) -> np.ndarray:
    x = np.ascontiguousarray(np.asarray(x), dtype=np.float32)
    assert x.shape == (B, N), x.shape
    nc = _get_nc()
    in_maps = [{"x": x[i]} for i in range(B)]
    res = run_bass_kernel_spmd(nc, in_maps, core_ids=list(range(B)))
    out = np.stack([np.asarray(res.results[i]["f0"]).reshape(N_OUT) for i in range(B)])
    return out.astype(np.float32)
